# revision 2
# baseline (speedup 1.0000x reference)
"""Trainium2 Bass kernel for Mixtral-style MoE (8 experts, top-2, SwiGLU).

Strategy: expert-parallel across the 8 NeuronCores with host-side dispatch.
The router is tiny (8192x2048x8 = 0.27 GFLOP) and runs on host CPU with the
exact same jax ops as the reference (bitwise-matching top-2 selection).  Each
core owns one expert: the host gathers that expert's routed tokens (avg 2048
of the 16384 (token, expert) pairs), pads to a uniform capacity C so all
cores run the same program (SPMD), and the device does only the expert FFN:

  h1 = W1 @ x ; h3 = W3 @ x ; h = silu(h1) * h3 ; y = (W2 @ h) * pair_weight

in bf16 with fp32 PSUM accumulation.  That is 2/8 of the dense-MoE FLOPs
(103 GFLOP/core vs 412 dense) -- the top-2 sparsity the dense baseline left
on the table.  The host scatter-adds each token's two expert partials.

Layouts (host-prepared, per core e):
  xg    : [H, C]  bf16  gathered tokens for expert e (token on free dim)
  wrow  : [1, C]  fp32  renormalized top-2 pair weight (0 on padding)
  w1t/w3t : [I/128, 128, H] bf16 blocked so the lhsT tile for k is a column
          slice of a contiguous [128, H] slab (slab i row p, col k*128+c
          holds w1[i*128+c, k*128+p], i.e. w1.T)
  w2t   : [H/128, 128, I] bf16, same blocking for w2.T
  out   : [H, C] fp32 partial outputs (host transposes/scatter-adds)
"""

import numpy as np
import ml_dtypes

import concourse.bass as bass
import concourse.mybir as mybir
import concourse.tile as tile
from concourse import bacc

P = 128
FP32 = mybir.dt.float32
BF16 = mybir.dt.bfloat16

# Full-problem constants
N_CORES = 8
NUM_TOKENS = 8192
HIDDEN = 2048
INTER = 4096
EXPERTS = 8
TOP_K = 2


def build_program(chunks, h=HIDDEN, i_sz=INTER):
    """chunks: tuple of token-chunk sizes (each <=512, mult of 128) summing
    to the per-core capacity C."""
    c_cap = sum(chunks)
    kt = h // P          # contraction tiles for GEMM1
    it = i_sz // P       # intermediate tiles
    ht = h // P          # output tiles

    nc = bacc.Bacc("TRN2", target_bir_lowering=False, debug=False)

    xg = nc.dram_tensor("xg", [h, c_cap], BF16, kind="ExternalInput").ap()
    wrow_d = nc.dram_tensor("wrow", [1, c_cap], FP32, kind="ExternalInput").ap()
    w1t = nc.dram_tensor("w1t", [it, P, h], BF16, kind="ExternalInput").ap()
    w3t = nc.dram_tensor("w3t", [it, P, h], BF16, kind="ExternalInput").ap()
    w2t = nc.dram_tensor("w2t", [ht, P, i_sz], BF16, kind="ExternalInput").ap()
    out_d = nc.dram_tensor("out", [h, c_cap], FP32, kind="ExternalOutput").ap()

    with tile.TileContext(nc) as tc:
        with (
            tc.tile_pool(name="const", bufs=1) as const_pool,
            tc.tile_pool(name="xpool", bufs=2) as x_pool,
            tc.tile_pool(name="hpool", bufs=1) as h_pool,
            tc.tile_pool(name="stream", bufs=3) as stream_pool,
            tc.tile_pool(name="w2stream", bufs=2) as w2_pool,
            tc.tile_pool(name="small", bufs=2) as small_pool,
            tc.tile_pool(name="work", bufs=2) as work_pool,
            tc.tile_pool(name="opool", bufs=3) as o_pool,
            tc.tile_pool(name="psum", bufs=2, space="PSUM") as psum_pool,
            tc.tile_pool(name="psum_s", bufs=2, space="PSUM") as psum_s,
        ):
            ones1 = const_pool.tile([1, P], FP32, tag="ones1")
            nc.vector.memset(ones1[:], 1.0)
            # pair-weight row resident for the whole kernel
            wrow = const_pool.tile([1, c_cap], FP32, tag="wrow")
            nc.sync.dma_start(out=wrow[:], in_=wrow_d[:, :])

            off = 0
            for ck in chunks:
                csl = slice(off, off + ck)

                # x tiles for this chunk
                xtb = []
                for k in range(kt):
                    x = x_pool.tile([P, ck], BF16, tag=f"xtb{k}")
                    nc.sync.dma_start(out=x[:], in_=xg[k * P:(k + 1) * P, csl])
                    xtb.append(x)

                # broadcast pair weights to [P, ck]
                wb_ps = psum_s.tile([P, ck], FP32, tag="wb")
                nc.tensor.matmul(out=wb_ps[:], lhsT=ones1[:],
                                 rhs=wrow[0:1, csl], start=True, stop=True)
                wbe = small_pool.tile([P, ck], FP32, tag="wbe")
                nc.vector.tensor_copy(out=wbe[:], in_=wb_ps[:])

                # GEMM1 + SwiGLU: h = silu(W1 x) * (W3 x), stored bf16
                h_sb = []
                for i in range(it):
                    w1s = stream_pool.tile([P, h], BF16, tag="w1s")
                    nc.sync.dma_start(out=w1s[:], in_=w1t[i])
                    w3s = stream_pool.tile([P, h], BF16, tag="w3s")
                    nc.sync.dma_start(out=w3s[:], in_=w3t[i])
                    h1_ps = psum_pool.tile([P, ck], FP32, tag="h1")
                    h3_ps = psum_pool.tile([P, ck], FP32, tag="h3")
                    for k in range(kt):
                        nc.tensor.matmul(out=h1_ps[:],
                                         lhsT=w1s[:, k * P:(k + 1) * P],
                                         rhs=xtb[k][:],
                                         start=(k == 0), stop=(k == kt - 1))
                    for k in range(kt):
                        nc.tensor.matmul(out=h3_ps[:],
                                         lhsT=w3s[:, k * P:(k + 1) * P],
                                         rhs=xtb[k][:],
                                         start=(k == 0), stop=(k == kt - 1))
                    sg = work_pool.tile([P, ck], FP32, tag="sg")
                    nc.scalar.activation(out=sg[:], in_=h1_ps[:],
                                         func=mybir.ActivationFunctionType.Sigmoid)
                    sil = work_pool.tile([P, ck], FP32, tag="sil")
                    nc.vector.tensor_tensor(out=sil[:], in0=sg[:], in1=h1_ps[:],
                                            op=mybir.AluOpType.mult)
                    hcur = h_pool.tile([P, ck], BF16, tag=f"h{i}")
                    nc.vector.tensor_tensor(out=hcur[:], in0=sil[:], in1=h3_ps[:],
                                            op=mybir.AluOpType.mult)
                    h_sb.append(hcur)

                # GEMM2: y = (W2 h) * pair_weight, streamed out
                for hh in range(ht):
                    w2s = w2_pool.tile([P, i_sz], BF16, tag="w2s")
                    nc.sync.dma_start(out=w2s[:], in_=w2t[hh])
                    f_ps = psum_pool.tile([P, ck], FP32, tag="f")
                    for i in range(it):
                        nc.tensor.matmul(out=f_ps[:],
                                         lhsT=w2s[:, i * P:(i + 1) * P],
                                         rhs=h_sb[i][:],
                                         start=(i == 0), stop=(i == it - 1))
                    yo = o_pool.tile([P, ck], FP32, tag="yo")
                    nc.vector.tensor_tensor(out=yo[:], in0=f_ps[:], in1=wbe[:],
                                            op=mybir.AluOpType.mult)
                    nc.sync.dma_start(out=out_d[hh * P:(hh + 1) * P, csl],
                                      in_=yo[:])
                off += ck

    nc.compile()
    return nc


# ---------------------------------------------------------------------------
# host side
# ---------------------------------------------------------------------------

def _block_w1_like(w):
    """[I, H] -> [I/128, 128, H] blocked so slab[i][p, k*128+c] =
    w[i*128+c, k*128+p] (i.e. w.T in lhsT-tile layout)."""
    i_sz, h = w.shape
    it = i_sz // P
    v = w.reshape(it, P, h // P, P)        # [i, c, k, p]
    return np.ascontiguousarray(v.transpose(0, 3, 2, 1)).reshape(it, P, h)


def _route(hs, gate):
    """Top-2 routing identical to the reference (jax on CPU)."""
    try:
        import jax
        import jax.numpy as jnp
        cpu = jax.devices("cpu")[0]
        with jax.default_device(cpu):
            logits = jnp.einsum('th,eh->te', jnp.asarray(hs), jnp.asarray(gate))
            probs = jax.nn.softmax(logits, axis=-1)
            topv, topi = jax.lax.top_k(probs, TOP_K)
            topv = topv / jnp.sum(topv, axis=-1, keepdims=True)
            return np.asarray(topi), np.asarray(topv, dtype=np.float32)
    except Exception:
        logits = hs.astype(np.float32) @ gate.astype(np.float32).T
        m = logits.max(axis=-1, keepdims=True)
        p = np.exp(logits - m)
        probs = p / p.sum(axis=-1, keepdims=True)
        topi = np.argsort(-probs, axis=-1, kind="stable")[:, :TOP_K]
        topv = np.take_along_axis(probs, topi, axis=-1)
        topv = topv / topv.sum(axis=-1, keepdims=True)
        return topi.astype(np.int64), topv.astype(np.float32)


_PROG_CACHE = {}


def _get_program(chunks=None):
    if chunks is None:
        chunks = _PROG_CACHE.get("last_key")
    if chunks not in _PROG_CACHE:
        _PROG_CACHE[chunks] = build_program(chunks)
    _PROG_CACHE["last_key"] = chunks
    return _PROG_CACHE[chunks]


def kernel(index, hidden_states, gate_w, w1, w3, w2, _trace=False):
    from concourse.bass_utils import run_bass_kernel_spmd

    idx = int(np.asarray(index))
    hs = np.asarray(hidden_states, dtype=np.float32)      # [T, H]
    t_num, h = hs.shape

    topi, topv = _route(hs, np.asarray(gate_w[idx], dtype=np.float32))
    flat_e = topi.ravel()                                  # [2T] pair expert
    flat_t = np.repeat(np.arange(t_num), TOP_K)            # [2T] pair token
    flat_w = topv.ravel().astype(np.float32)               # [2T] pair weight

    counts = np.bincount(flat_e, minlength=EXPERTS)
    order = np.argsort(flat_e, kind="stable")
    ranks = np.empty_like(order)
    ranks[order] = np.arange(order.size)
    starts = np.concatenate([[0], np.cumsum(counts)])
    within = ranks - starts[flat_e]                        # rank inside expert
    cmax = max(int(counts.max()), 256)
    c_cap = -(-cmax // 256) * 256
    chunks = (512,) * (c_cap // 512) + ((256,) if c_cap % 512 else ())

    nc = _get_program(chunks)

    hs_bf = np.asarray(hs, dtype=ml_dtypes.bfloat16)
    in_maps = []
    for e in range(EXPERTS):
        sel = order[starts[e]:starts[e + 1]]
        tok = flat_t[sel]
        xpad = np.zeros((c_cap, h), dtype=ml_dtypes.bfloat16)
        xpad[:tok.size] = hs_bf[tok]
        wr = np.zeros((1, c_cap), dtype=np.float32)
        wr[0, :tok.size] = flat_w[sel]
        in_maps.append({
            "xg": np.ascontiguousarray(xpad.T),
            "wrow": wr,
            "w1t": _block_w1_like(np.asarray(w1[idx, e], dtype=ml_dtypes.bfloat16)),
            "w3t": _block_w1_like(np.asarray(w3[idx, e], dtype=ml_dtypes.bfloat16)),
            "w2t": _block_w1_like(np.asarray(w2[idx, e], dtype=ml_dtypes.bfloat16)),
        })

    res = run_bass_kernel_spmd(nc, in_maps, core_ids=list(range(N_CORES)),
                               trace=False)
    # y_all[e*c_cap + r] = output row (length H) of pair with rank r in expert e
    y_all = np.concatenate(
        [np.asarray(r["out"], dtype=np.float32).T for r in res.results], axis=0)
    pos = flat_e * c_cap + within                          # [2T]
    out = y_all[pos[0::2]] + y_all[pos[1::2]]
    kernel._last_in_maps = in_maps
    return out


# revision 4
# speedup vs baseline: 1.1920x; 1.1920x over previous
"""Trainium2 Bass kernel for Mixtral-style MoE (8 experts, top-2, SwiGLU).

Strategy: expert-parallel across the 8 NeuronCores with host-side dispatch.
The router is tiny (8192x2048x8 = 0.27 GFLOP) and runs on host CPU with the
exact same jax ops as the reference (bitwise-matching top-2 selection).  Each
core owns one expert: the host gathers that expert's routed tokens (avg 2048
of the 16384 (token, expert) pairs), pads to a uniform capacity C so all
cores run the same program (SPMD), and the device does only the expert FFN:

  h1 = W1 @ x ; h3 = W3 @ x ; h = silu(h1) * h3 ; y = (W2 @ h) * pair_weight

in bf16 with fp32 PSUM accumulation.  That is 2/8 of the dense-MoE FLOPs
(103 GFLOP/core vs 412 dense).  The host scatter-adds each token's two
expert partials.

Token chunks are processed in PAIRS per weight pass: each streamed weight
tile issues two back-to-back matmuls (chunk A, chunk B) so the stationary-
operand load amortizes over 1024 moving columns instead of 512, and weight
HBM traffic halves.

Layouts (host-prepared, per core e):
  xg    : [H, C]  bf16  gathered tokens for expert e (token on free dim)
  wrow  : [1, C]  fp32  renormalized top-2 pair weight (0 on padding)
  w1t/w3t : [I/128, 128, H] bf16 blocked so the lhsT tile for k is a column
          slice of a contiguous [128, H] slab (slab i row p, col k*128+c
          holds w1[i*128+c, k*128+p], i.e. w1.T)
  w2t   : [H/128, 128, I] bf16, same blocking for w2.T
  out   : [H, C] fp32 partial outputs (host transposes/scatter-adds)
"""

import numpy as np
import ml_dtypes

import concourse.bass as bass
import concourse.mybir as mybir
import concourse.tile as tile
from concourse import bacc

P = 128
FP32 = mybir.dt.float32
BF16 = mybir.dt.bfloat16

# Full-problem constants
N_CORES = 8
NUM_TOKENS = 8192
HIDDEN = 2048
INTER = 4096
EXPERTS = 8
TOP_K = 2


def build_program(groups, h=HIDDEN, i_sz=INTER):
    """groups: tuple of tuples of chunk sizes, e.g. ((512,512),(512,512),(128,)).
    Chunks in one group share each streamed weight tile (paired matmuls)."""
    c_cap = sum(sum(g) for g in groups)
    kt = h // P          # contraction tiles for GEMM1
    it = i_sz // P       # intermediate tiles
    ht = h // P          # output tiles

    nc = bacc.Bacc("TRN2", target_bir_lowering=False, debug=False)

    xg = nc.dram_tensor("xg", [h, c_cap], BF16, kind="ExternalInput").ap()
    wrow_d = nc.dram_tensor("wrow", [1, c_cap], FP32, kind="ExternalInput").ap()
    w1t = nc.dram_tensor("w1t", [it, P, h], BF16, kind="ExternalInput").ap()
    w3t = nc.dram_tensor("w3t", [it, P, h], BF16, kind="ExternalInput").ap()
    w2t = nc.dram_tensor("w2t", [ht, P, i_sz], BF16, kind="ExternalInput").ap()
    out_d = nc.dram_tensor("out", [h, c_cap], FP32, kind="ExternalOutput").ap()

    with tile.TileContext(nc) as tc:
        with (
            tc.tile_pool(name="const", bufs=1) as const_pool,
            tc.tile_pool(name="xpool", bufs=1) as x_pool,
            tc.tile_pool(name="hpool", bufs=1) as h_pool,
            tc.tile_pool(name="stream", bufs=3) as stream_pool,
            tc.tile_pool(name="w2stream", bufs=2) as w2_pool,
            tc.tile_pool(name="small", bufs=2) as small_pool,
            tc.tile_pool(name="work", bufs=2) as work_pool,
            tc.tile_pool(name="opool", bufs=3) as o_pool,
            tc.tile_pool(name="psum1", bufs=1, space="PSUM") as psum1,
            tc.tile_pool(name="psum2", bufs=2, space="PSUM") as psum2,
        ):
            ones1 = const_pool.tile([1, P], FP32, tag="ones1")
            nc.vector.memset(ones1[:], 1.0)
            # pair-weight row resident for the whole kernel
            wrow = const_pool.tile([1, c_cap], FP32, tag="wrow")
            nc.sync.dma_start(out=wrow[:], in_=wrow_d[:, :])

            off = 0
            for grp in groups:
                cks = []                       # [(offset, size), ...]
                for ck in grp:
                    cks.append((off, ck))
                    off += ck
                nch = len(cks)

                # x tiles + broadcast pair weights per chunk
                xtb = []                       # [c][k]
                wbe = []                       # [c]
                for c, (o, ck) in enumerate(cks):
                    row = []
                    for k in range(kt):
                        x = x_pool.tile([P, ck], BF16, tag=f"xtb{c}_{k}")
                        nc.sync.dma_start(out=x[:],
                                          in_=xg[k * P:(k + 1) * P, o:o + ck])
                        row.append(x)
                    xtb.append(row)
                    wb_ps = psum2.tile([P, ck], FP32, tag="f_0")
                    nc.tensor.matmul(out=wb_ps[:], lhsT=ones1[:],
                                     rhs=wrow[0:1, o:o + ck],
                                     start=True, stop=True)
                    wb = small_pool.tile([P, ck], FP32, tag=f"wbe{c}")
                    nc.vector.tensor_copy(out=wb[:], in_=wb_ps[:])
                    wbe.append(wb)

                # GEMM1 + SwiGLU: h = silu(W1 x) * (W3 x), stored bf16
                h_sb = [[] for _ in range(nch)]
                for i in range(it):
                    w1s = stream_pool.tile([P, h], BF16, tag="w1s")
                    nc.sync.dma_start(out=w1s[:], in_=w1t[i])
                    w3s = stream_pool.tile([P, h], BF16, tag="w3s")
                    nc.sync.dma_start(out=w3s[:], in_=w3t[i])
                    h1_ps = [psum1.tile([P, ck], FP32, tag=f"h1_{c}", name=f"h1_{c}")
                             for c, (_, ck) in enumerate(cks)]
                    h3_ps = [psum1.tile([P, ck], FP32, tag=f"h3_{c}", name=f"h3_{c}")
                             for c, (_, ck) in enumerate(cks)]
                    for k in range(kt):
                        for c in range(nch):
                            nc.tensor.matmul(out=h1_ps[c][:],
                                             lhsT=w1s[:, k * P:(k + 1) * P],
                                             rhs=xtb[c][k][:],
                                             start=(k == 0), stop=(k == kt - 1))
                    for k in range(kt):
                        for c in range(nch):
                            nc.tensor.matmul(out=h3_ps[c][:],
                                             lhsT=w3s[:, k * P:(k + 1) * P],
                                             rhs=xtb[c][k][:],
                                             start=(k == 0), stop=(k == kt - 1))
                    for c, (_, ck) in enumerate(cks):
                        sg = work_pool.tile([P, ck], FP32, tag=f"sg{c}")
                        nc.scalar.activation(
                            out=sg[:], in_=h1_ps[c][:],
                            func=mybir.ActivationFunctionType.Sigmoid)
                        sil = work_pool.tile([P, ck], FP32, tag=f"sil{c}")
                        nc.vector.tensor_tensor(out=sil[:], in0=sg[:],
                                                in1=h1_ps[c][:],
                                                op=mybir.AluOpType.mult)
                        hcur = h_pool.tile([P, ck], BF16, tag=f"h{c}_{i}")
                        nc.vector.tensor_tensor(out=hcur[:], in0=sil[:],
                                                in1=h3_ps[c][:],
                                                op=mybir.AluOpType.mult)
                        h_sb[c].append(hcur)

                # GEMM2: y = (W2 h) * pair_weight, streamed out
                for hh in range(ht):
                    w2s = w2_pool.tile([P, i_sz], BF16, tag="w2s")
                    nc.sync.dma_start(out=w2s[:], in_=w2t[hh])
                    f_ps = [psum2.tile([P, ck], FP32, tag=f"f_{c}", name=f"f_{c}")
                            for c, (_, ck) in enumerate(cks)]
                    for i in range(it):
                        for c in range(nch):
                            nc.tensor.matmul(out=f_ps[c][:],
                                             lhsT=w2s[:, i * P:(i + 1) * P],
                                             rhs=h_sb[c][i][:],
                                             start=(i == 0), stop=(i == it - 1))
                    for c, (o, ck) in enumerate(cks):
                        yo = o_pool.tile([P, ck], FP32, tag=f"yo{c}")
                        nc.vector.tensor_tensor(out=yo[:], in0=f_ps[c][:],
                                                in1=wbe[c][:],
                                                op=mybir.AluOpType.mult)
                        nc.sync.dma_start(out=out_d[hh * P:(hh + 1) * P,
                                                    o:o + ck],
                                          in_=yo[:])

    nc.compile()
    return nc


# ---------------------------------------------------------------------------
# host side
# ---------------------------------------------------------------------------

def _block_w1_like(w):
    """[I, H] -> [I/128, 128, H] blocked so slab[i][p, k*128+c] =
    w[i*128+c, k*128+p] (i.e. w.T in lhsT-tile layout)."""
    i_sz, h = w.shape
    it = i_sz // P
    v = w.reshape(it, P, h // P, P)        # [i, c, k, p]
    return np.ascontiguousarray(v.transpose(0, 3, 2, 1)).reshape(it, P, h)


def _route(hs, gate):
    """Top-2 routing identical to the reference (jax on CPU)."""
    try:
        import jax
        import jax.numpy as jnp
        cpu = jax.devices("cpu")[0]
        with jax.default_device(cpu):
            logits = jnp.einsum('th,eh->te', jnp.asarray(hs), jnp.asarray(gate))
            probs = jax.nn.softmax(logits, axis=-1)
            topv, topi = jax.lax.top_k(probs, TOP_K)
            topv = topv / jnp.sum(topv, axis=-1, keepdims=True)
            return np.asarray(topi), np.asarray(topv, dtype=np.float32)
    except Exception:
        logits = hs.astype(np.float32) @ gate.astype(np.float32).T
        m = logits.max(axis=-1, keepdims=True)
        p = np.exp(logits - m)
        probs = p / p.sum(axis=-1, keepdims=True)
        topi = np.argsort(-probs, axis=-1, kind="stable")[:, :TOP_K]
        topv = np.take_along_axis(probs, topi, axis=-1)
        topv = topv / topv.sum(axis=-1, keepdims=True)
        return topi.astype(np.int64), topv.astype(np.float32)


def _make_groups(c_cap):
    """Pair 512-token chunks; tail (<1024) becomes single chunks <=512."""
    groups = []
    rem = c_cap
    while rem >= 1024:
        groups.append((512, 512))
        rem -= 1024
    while rem > 0:
        ck = min(rem, 512)
        groups.append((ck,))
        rem -= ck
    return tuple(groups)


_PROG_CACHE = {}


def _get_program(groups=None):
    if groups is None:
        groups = _PROG_CACHE.get("last_key")
    if groups not in _PROG_CACHE:
        _PROG_CACHE[groups] = build_program(groups)
    _PROG_CACHE["last_key"] = groups
    return _PROG_CACHE[groups]


def kernel(index, hidden_states, gate_w, w1, w3, w2, _trace=False):
    from concourse.bass_utils import run_bass_kernel_spmd

    idx = int(np.asarray(index))
    hs = np.asarray(hidden_states, dtype=np.float32)      # [T, H]
    t_num, h = hs.shape

    topi, topv = _route(hs, np.asarray(gate_w[idx], dtype=np.float32))
    flat_e = topi.ravel()                                  # [2T] pair expert
    flat_t = np.repeat(np.arange(t_num), TOP_K)            # [2T] pair token
    flat_w = topv.ravel().astype(np.float32)               # [2T] pair weight

    counts = np.bincount(flat_e, minlength=EXPERTS)
    order = np.argsort(flat_e, kind="stable")
    ranks = np.empty_like(order)
    ranks[order] = np.arange(order.size)
    starts = np.concatenate([[0], np.cumsum(counts)])
    within = ranks - starts[flat_e]                        # rank inside expert
    cmax = max(int(counts.max()), 128)
    c_cap = -(-cmax // 128) * 128
    groups = _make_groups(c_cap)

    nc = _get_program(groups)

    hs_bf = np.asarray(hs, dtype=ml_dtypes.bfloat16)
    in_maps = []
    for e in range(EXPERTS):
        sel = order[starts[e]:starts[e + 1]]
        tok = flat_t[sel]
        xpad = np.zeros((c_cap, h), dtype=ml_dtypes.bfloat16)
        xpad[:tok.size] = hs_bf[tok]
        wr = np.zeros((1, c_cap), dtype=np.float32)
        wr[0, :tok.size] = flat_w[sel]
        in_maps.append({
            "xg": np.ascontiguousarray(xpad.T),
            "wrow": wr,
            "w1t": _block_w1_like(np.asarray(w1[idx, e], dtype=ml_dtypes.bfloat16)),
            "w3t": _block_w1_like(np.asarray(w3[idx, e], dtype=ml_dtypes.bfloat16)),
            "w2t": _block_w1_like(np.asarray(w2[idx, e], dtype=ml_dtypes.bfloat16)),
        })

    res = run_bass_kernel_spmd(nc, in_maps, core_ids=list(range(N_CORES)),
                               trace=False)
    # y_all[e*c_cap + r] = output row (length H) of pair with rank r in expert e
    y_all = np.concatenate(
        [np.asarray(r["out"], dtype=np.float32).T for r in res.results], axis=0)
    pos = flat_e * c_cap + within                          # [2T]
    out = y_all[pos[0::2]] + y_all[pos[1::2]]
    kernel._last_in_maps = in_maps
    return out


# revision 10
# speedup vs baseline: 1.3377x; 1.1222x over previous
"""Trainium2 Bass kernel for Mixtral-style MoE (8 experts, top-2, SwiGLU).

Strategy: expert-parallel across the 8 NeuronCores with host-side dispatch.
The router is tiny (8192x2048x8 = 0.27 GFLOP) and runs on host CPU with the
exact same jax ops as the reference (bitwise-matching top-2 selection).  Each
core owns one expert: the host gathers that expert's routed tokens (avg 2048
of the 16384 (token, expert) pairs), pads to a uniform capacity C so all
cores run the same program (SPMD), and the device does only the expert FFN:

  h1 = W1 @ x ; h3 = W3 @ x ; h = silu(h1) * h3 ; y = (W2 @ h) * pair_weight

in bf16 with fp32 PSUM accumulation.  That is 2/8 of the dense-MoE FLOPs
(103 GFLOP/core vs 412 dense).  The host scatter-adds each token's two
expert partials.

Token chunks are processed in PAIRS per weight pass: each streamed weight
tile issues two back-to-back matmuls (chunk A, chunk B) so the stationary-
operand load amortizes over 1024 moving columns instead of 512, and weight
HBM traffic halves.

Layouts (host-prepared, per core e):
  xg    : [H, C]  bf16  gathered tokens for expert e (token on free dim)
  wrow  : [1, C]  fp32  renormalized top-2 pair weight (0 on padding)
  w1t/w3t : [I/128, 128, H] bf16 blocked so the lhsT tile for k is a column
          slice of a contiguous [128, H] slab (slab i row p, col k*128+c
          holds w1[i*128+c, k*128+p], i.e. w1.T)
  w2t   : [H/128, 128, I] bf16, same blocking for w2.T
  out   : [H, C] fp32 partial outputs (host transposes/scatter-adds)
"""

import numpy as np
import ml_dtypes

import concourse.bass as bass
import concourse.mybir as mybir
import concourse.tile as tile
from concourse import bacc

P = 128
FP32 = mybir.dt.float32
BF16 = mybir.dt.bfloat16

# Full-problem constants
N_CORES = 8
NUM_TOKENS = 8192
HIDDEN = 2048
INTER = 4096
EXPERTS = 8
TOP_K = 2


def build_program(groups, h=HIDDEN, i_sz=INTER):
    """groups: tuple of tuples of chunk sizes.  Each group is either
    (a,) / (a, b) with a,b <= 512, or (a, b, t) with t <= 256 (tail rider).
    """
    c_cap = sum(sum(g) for g in groups)
    kt = h // P
    it = i_sz // P
    ht = h // P

    nc = bacc.Bacc("TRN2", target_bir_lowering=False, debug=False)

    xg = nc.dram_tensor("xg", [h, c_cap], BF16, kind="ExternalInput").ap()
    wrow_d = nc.dram_tensor("wrow", [1, c_cap], FP32, kind="ExternalInput").ap()
    w1t = nc.dram_tensor("w1t", [it, P, h], BF16, kind="ExternalInput").ap()
    w3t = nc.dram_tensor("w3t", [it, P, h], BF16, kind="ExternalInput").ap()
    w2t = nc.dram_tensor("w2t", [ht, P, i_sz], BF16, kind="ExternalInput").ap()
    out_d = nc.dram_tensor("out", [h, c_cap], FP32, kind="ExternalOutput").ap()

    with tile.TileContext(nc) as tc:
        with (
            tc.tile_pool(name="const", bufs=1) as const_pool,
            tc.tile_pool(name="xpool", bufs=1) as x_pool,
            tc.tile_pool(name="hpool", bufs=1) as h_pool,
            tc.tile_pool(name="stream", bufs=3) as stream_pool,
            tc.tile_pool(name="w2stream", bufs=2) as w2_pool,
            tc.tile_pool(name="work", bufs=2) as work_pool,
            tc.tile_pool(name="opool", bufs=3) as o_pool,
            tc.tile_pool(name="psum1", bufs=1, space="PSUM") as psum1,
            tc.tile_pool(name="psum2", bufs=1, space="PSUM") as psum2,
        ):
            ones1 = const_pool.tile([1, P], FP32, tag="ones1")
            nc.vector.memset(ones1[:], 1.0)
            wrow = const_pool.tile([1, c_cap], FP32, tag="wrow")
            nc.scalar.dma_start(out=wrow[:], in_=wrow_d[:, :])
            # broadcast pair weights once for the whole capacity
            wb_full = const_pool.tile([P, c_cap], FP32, tag="wb_full")
            for o in range(0, c_cap, 512):
                ck = min(512, c_cap - o)
                wb_ps = psum2.tile([P, ck], FP32, tag="f_0", name="wb_ps")
                nc.tensor.matmul(out=wb_ps[:], lhsT=ones1[:],
                                 rhs=wrow[0:1, o:o + ck], start=True, stop=True)
                nc.vector.tensor_copy(out=wb_full[:, o:o + ck], in_=wb_ps[:])

            off = 0
            for grp in groups:
                cks = []
                for ck in grp:
                    cks.append((off, ck))
                    off += ck
                nch = len(cks)
                has_tail = nch == 3
                if has_tail:
                    assert cks[2][1] <= 256

                # x tiles, k-major to match matmul consumption order
                xtb = [[None] * kt for _ in range(nch)]
                for k in range(kt):
                    for c, (o, ck) in enumerate(cks):
                        x = x_pool.tile([P, ck], BF16, tag=f"xtb{c}_{k}",
                                        name=f"xtb{c}_{k}")
                        nc.scalar.dma_start(
                            out=x[:], in_=xg[k * P:(k + 1) * P, o:o + ck])
                        xtb[c][k] = x

                # GEMM1 + SwiGLU
                h_sb = [[] for _ in range(nch)]
                for i in range(it):
                    w1s = stream_pool.tile([P, h], BF16, tag="w1s")
                    nc.sync.dma_start(out=w1s[:], in_=w1t[i])
                    w3s = stream_pool.tile([P, h], BF16, tag="w3s")
                    nc.sync.dma_start(out=w3s[:], in_=w3t[i])
                    h1_ps, h3_ps = [], []
                    for c, (_, ck) in enumerate(cks):
                        if c == 2:  # tail rider: one bank, two halves
                            hT = psum1.tile([P, 2 * ck], FP32, tag="hT",
                                            name="hT")
                            h1_ps.append(hT[:, 0:ck])
                            h3_ps.append(hT[:, ck:2 * ck])
                        else:
                            t1 = psum1.tile([P, ck], FP32, tag=f"h1_{c}",
                                            name=f"h1_{c}")
                            t3 = psum1.tile([P, ck], FP32, tag=f"h3_{c}",
                                            name=f"h3_{c}")
                            h1_ps.append(t1[:])
                            h3_ps.append(t3[:])
                    for k in range(kt):
                        for c in range(nch):
                            nc.tensor.matmul(out=h1_ps[c],
                                             lhsT=w1s[:, k * P:(k + 1) * P],
                                             rhs=xtb[c][k][:],
                                             start=(k == 0), stop=(k == kt - 1))
                    for k in range(kt):
                        for c in range(nch):
                            nc.tensor.matmul(out=h3_ps[c],
                                             lhsT=w3s[:, k * P:(k + 1) * P],
                                             rhs=xtb[c][k][:],
                                             start=(k == 0), stop=(k == kt - 1))
                    for c, (_, ck) in enumerate(cks):
                        sg = work_pool.tile([P, ck], FP32, tag=f"sg{c}",
                                            name=f"sg{c}")
                        nc.scalar.activation(
                            out=sg[:], in_=h1_ps[c],
                            func=mybir.ActivationFunctionType.Sigmoid)
                        sil = work_pool.tile([P, ck], FP32, tag=f"sil{c}",
                                             name=f"sil{c}")
                        nc.vector.tensor_tensor(out=sil[:], in0=sg[:],
                                                in1=h1_ps[c],
                                                op=mybir.AluOpType.mult)
                        hcur = h_pool.tile([P, ck], BF16, tag=f"h{c}_{i}",
                                           name=f"h{c}_{i}")
                        nc.vector.tensor_tensor(out=hcur[:], in0=sil[:],
                                                in1=h3_ps[c],
                                                op=mybir.AluOpType.mult)
                        h_sb[c].append(hcur)

                # GEMM2
                for hh in range(ht):
                    w2s = w2_pool.tile([P, i_sz], BF16, tag="w2s")
                    nc.sync.dma_start(out=w2s[:], in_=w2t[hh])
                    f_ps = [psum2.tile([P, ck], FP32, tag=f"f_{c}",
                                       name=f"f_{c}")
                            for c, (_, ck) in enumerate(cks)]
                    for i in range(it):
                        for c in range(nch):
                            nc.tensor.matmul(out=f_ps[c][:],
                                             lhsT=w2s[:, i * P:(i + 1) * P],
                                             rhs=h_sb[c][i][:],
                                             start=(i == 0), stop=(i == it - 1))
                    for c, (o, ck) in enumerate(cks):
                        yo = o_pool.tile([P, ck], FP32, tag=f"yo{c}",
                                         name=f"yo{c}")
                        nc.vector.tensor_tensor(out=yo[:], in0=f_ps[c][:],
                                                in1=wb_full[:, o:o + ck],
                                                op=mybir.AluOpType.mult)
                        nc.scalar.dma_start(
                            out=out_d[hh * P:(hh + 1) * P, o:o + ck],
                            in_=yo[:])

    nc.compile()
    return nc


# ---------------------------------------------------------------------------
# host side
# ---------------------------------------------------------------------------

def _block_w1_like(w):
    """[I, H] -> [I/128, 128, H] blocked so slab[i][p, k*128+c] =
    w[i*128+c, k*128+p] (i.e. w.T in lhsT-tile layout)."""
    i_sz, h = w.shape
    it = i_sz // P
    v = w.reshape(it, P, h // P, P)        # [i, c, k, p]
    return np.ascontiguousarray(v.transpose(0, 3, 2, 1)).reshape(it, P, h)


def _route(hs, gate):
    """Top-2 routing identical to the reference (jax on CPU)."""
    try:
        import jax
        import jax.numpy as jnp
        cpu = jax.devices("cpu")[0]
        with jax.default_device(cpu):
            logits = jnp.einsum('th,eh->te', jnp.asarray(hs), jnp.asarray(gate))
            probs = jax.nn.softmax(logits, axis=-1)
            topv, topi = jax.lax.top_k(probs, TOP_K)
            topv = topv / jnp.sum(topv, axis=-1, keepdims=True)
            return np.asarray(topi), np.asarray(topv, dtype=np.float32)
    except Exception:
        logits = hs.astype(np.float32) @ gate.astype(np.float32).T
        m = logits.max(axis=-1, keepdims=True)
        p = np.exp(logits - m)
        probs = p / p.sum(axis=-1, keepdims=True)
        topi = np.argsort(-probs, axis=-1, kind="stable")[:, :TOP_K]
        topv = np.take_along_axis(probs, topi, axis=-1)
        topv = topv / topv.sum(axis=-1, keepdims=True)
        return topi.astype(np.int64), topv.astype(np.float32)


def _make_groups(c_cap):
    """Pair 512-chunks; leftover (<1024) rides as <=256 third chunks on the
    pair groups; any remainder beyond that becomes single-chunk groups."""
    n_pairs, rem = divmod(c_cap, 1024)
    groups = [[512, 512] for _ in range(n_pairs)]
    gi = 0
    while rem > 0 and gi < len(groups):
        t = min(rem, 256)
        groups[gi].append(t)
        rem -= t
        gi += 1
    while rem > 0:  # no pair groups to ride on
        ck = min(rem, 512)
        groups.append([ck])
        rem -= ck
    return tuple(tuple(g) for g in groups)


_PROG_CACHE = {}


def _get_program(groups=None):
    if groups is None:
        groups = _PROG_CACHE.get("last_key")
    if groups not in _PROG_CACHE:
        _PROG_CACHE[groups] = build_program(groups)
    _PROG_CACHE["last_key"] = groups
    return _PROG_CACHE[groups]


def kernel(index, hidden_states, gate_w, w1, w3, w2, _trace=False):
    from concourse.bass_utils import run_bass_kernel_spmd

    idx = int(np.asarray(index))
    hs = np.asarray(hidden_states, dtype=np.float32)      # [T, H]
    t_num, h = hs.shape

    topi, topv = _route(hs, np.asarray(gate_w[idx], dtype=np.float32))
    flat_e = topi.ravel()                                  # [2T] pair expert
    flat_t = np.repeat(np.arange(t_num), TOP_K)            # [2T] pair token
    flat_w = topv.ravel().astype(np.float32)               # [2T] pair weight

    counts = np.bincount(flat_e, minlength=EXPERTS)
    order = np.argsort(flat_e, kind="stable")
    ranks = np.empty_like(order)
    ranks[order] = np.arange(order.size)
    starts = np.concatenate([[0], np.cumsum(counts)])
    within = ranks - starts[flat_e]                        # rank inside expert
    # Device capacity caps at 2048 (clean 2x(512,512) chunk groups); the few
    # overflow pairs beyond an expert's first 2048 (capacity-factor spill)
    # are computed on host in fp32.
    c_cap = min(max(int(counts.max()), 128), 2048)
    groups = _make_groups(c_cap)

    nc = _get_program(groups)

    hs_bf = np.asarray(hs, dtype=ml_dtypes.bfloat16)
    in_maps = []
    for e in range(EXPERTS):
        sel = order[starts[e]:starts[e + 1]][:c_cap]
        tok = flat_t[sel]
        xpad = np.zeros((c_cap, h), dtype=ml_dtypes.bfloat16)
        xpad[:tok.size] = hs_bf[tok]
        wr = np.zeros((1, c_cap), dtype=np.float32)
        wr[0, :tok.size] = flat_w[sel]
        in_maps.append({
            "xg": np.ascontiguousarray(xpad.T),
            "wrow": wr,
            "w1t": _block_w1_like(np.asarray(w1[idx, e], dtype=ml_dtypes.bfloat16)),
            "w3t": _block_w1_like(np.asarray(w3[idx, e], dtype=ml_dtypes.bfloat16)),
            "w2t": _block_w1_like(np.asarray(w2[idx, e], dtype=ml_dtypes.bfloat16)),
        })

    res = run_bass_kernel_spmd(nc, in_maps, core_ids=list(range(N_CORES)),
                               trace=False)
    # y_all[e*c_cap + r] = output row (length H) of pair with rank r in expert e
    y_all = np.concatenate(
        [np.asarray(r["out"], dtype=np.float32).T for r in res.results], axis=0)
    pos = flat_e * c_cap + within                          # [2T]
    ovf = within >= c_cap                                  # capacity spill
    contrib = np.empty((pos.size, h), dtype=np.float32)
    contrib[~ovf] = y_all[pos[~ovf]]
    if ovf.any():
        for e in np.unique(flat_e[ovf]):
            m = ovf & (flat_e == e)
            x_e = hs[flat_t[m]]                            # [n, H] fp32
            h1 = x_e @ np.asarray(w1[idx, e], dtype=np.float32).T
            h3 = x_e @ np.asarray(w3[idx, e], dtype=np.float32).T
            hsw = (h1 / (1.0 + np.exp(-h1))) * h3
            contrib[m] = (hsw @ np.asarray(w2[idx, e], dtype=np.float32).T
                          ) * flat_w[m][:, None]
    out = contrib[0::2] + contrib[1::2]
    kernel._last_in_maps = in_maps
    return out
